# revision 1
# baseline (speedup 1.0000x reference)
"""CTLSTM cell fused kernel for 8 Trainium2 NeuronCores.

Strategy (data-parallel over batch):
  - B=16384 rows sharded 2048/core; weights replicated.
  - Host stages transposed operands so the K contraction dim lands on SBUF
    partitions: xh = [x;ht].T -> [1024, 2048/core], w2 = [Wx;Wh].T ->
    [1024, 3584], both cast to bf16 (PE runs 1 col/cycle and FWL hides the
    weight loads; fp32 would serialize a ~190ns LDWEIGHTS per matmul).
    PSUM accumulation stays fp32.
  - Gate columns are host-permuted to [z, d, i, f, o, i_bar, f_bar] so the
    five sigmoid gates are contiguous: per 128-row subtile ACT runs one
    tanh, one sigmoid(-x) and ONE [128,2560] sigmoid, all in place in a
    contiguous [128,3584] pre-activation mega-tile.
  - bf16 allows N=1024 moving: matmuls compute gate PAIRS into 2-bank
    PSUM tiles; DVE drains each pair with a single fused bias-add.
  - softplus(wd) has no ACT table set; computed as -ln(sigmoid(-wd)).
    sigmoid(-wd) from the main pass is stashed in SBUF; Ln chunks at the
    end are forced (explicit deps) after all main-pass ACT ops so the
    activation table switches exactly once.
"""

import numpy as np
import ml_dtypes

import concourse.bacc as bacc
import concourse.bass as bass
import concourse.mybir as mybir
import concourse.tile as tile
from concourse.tile_rust import add_dep_helper
from concourse.bass_utils import run_bass_kernel_spmd

NCORES = 8
B = 16384
I = 512
H = 512
NG = 7
G = NG * H          # 3584
K2 = I + H          # 1024
P = 128
BS = B // NCORES    # 2048 rows per core
NT = BS // P        # 16 subtiles of 128 rows
SUP = 4             # subtiles per supertile (DMA granularity)
NSUP = NT // SUP

BF16 = mybir.dt.bfloat16
F32 = mybir.dt.float32
AF = mybir.ActivationFunctionType
NPBF16 = ml_dtypes.bfloat16

# gate order in the permuted weight/bias layout (reference order is
# i, f, z, o, d, i_bar, f_bar)
PERM = [2, 4, 0, 1, 3, 5, 6]   # -> z, d, i, f, o, i_bar, f_bar

TRACE = False
LAST_RESULTS = None

_nc_cache = None


def _build():
    nc = bacc.Bacc("TRN2", target_bir_lowering=False, debug=False)

    xh = nc.dram_tensor("xh", [K2, BS], BF16, kind="ExternalInput")
    w2 = nc.dram_tensor("w2", [K2, G], BF16, kind="ExternalInput")
    ct = nc.dram_tensor("ct", [BS, H], F32, kind="ExternalInput")
    bb_d = nc.dram_tensor("bb", [P, G], F32, kind="ExternalInput")

    h_d = nc.dram_tensor("h", [BS, H], F32, kind="ExternalOutput")
    c_d = nc.dram_tensor("c", [BS, H], F32, kind="ExternalOutput")
    cb_d = nc.dram_tensor("cb", [BS, H], F32, kind="ExternalOutput")
    o_d = nc.dram_tensor("o", [BS, H], F32, kind="ExternalOutput")
    dr_d = nc.dram_tensor("dr", [BS, H], F32, kind="ExternalOutput")

    last_sn = None  # final main-pass ACT instruction, gates phase 2

    with tile.TileContext(nc) as tc:
        with (
            tc.tile_pool(name="wp", bufs=1) as wp,
            tc.tile_pool(name="cp", bufs=1) as cp,
            tc.tile_pool(name="sp", bufs=1) as sp,
            tc.tile_pool(name="xp", bufs=2) as xp,
            tc.tile_pool(name="ctp", bufs=4) as ctp,
            tc.tile_pool(name="gp", bufs=2) as gp,
            tc.tile_pool(name="pp", bufs=3, space=bass.MemorySpace.PSUM) as pp,
            tc.tile_pool(name="pps", bufs=2, space=bass.MemorySpace.PSUM) as pps,
        ):
            # resident weights: 8 K-chunks of [128, 3584] bf16
            w_sb = []
            for k in range(8):
                wt = wp.tile([P, G], BF16, tag=f"w{k}")
                nc.sync.dma_start(wt[:], w2[k * P:(k + 1) * P, :])
                w_sb.append(wt)
            # broadcast bias [128, 3584] fp32 (bx+bh, host-staged broadcast)
            bb = cp.tile([P, G], F32, tag="bb")
            nc.sync.dma_start(bb[:], bb_d[:])
            # sigmoid(-wd) stash, one [128, 512] slice per subtile
            stash = sp.tile([P, NT, H], F32, tag="stash")

            for s in range(NSUP):
                xhs = []
                for k in range(8):
                    t_ = xp.tile([P, SUP * P], BF16, tag=f"xh{k}")
                    nc.sync.dma_start(
                        t_[:], xh[k * P:(k + 1) * P, s * SUP * P:(s + 1) * SUP * P]
                    )
                    xhs.append(t_)

                for j in range(SUP):
                    t = s * SUP + j
                    bsl = slice(j * P, (j + 1) * P)
                    rows = slice(t * P, (t + 1) * P)

                    ctj = ctp.tile([P, H], F32, tag="ct")
                    nc.sync.dma_start(ctj[:], ct[rows, :])

                    ga = gp.tile([P, G], F32, tag="ga")

                    # gate pairs (z,d), (i,f), (o,ib) then single (fb); each
                    # pair accumulates in a 2-bank PSUM tile drained by one
                    # fused bias-add
                    for pr in range(3):
                        csl = slice(pr * 2 * H, (pr + 1) * 2 * H)
                        acc = pp.tile([P, 2 * H], F32, tag="accp")
                        for half in range(2):
                            gsl = slice((pr * 2 + half) * H,
                                        (pr * 2 + half + 1) * H)
                            hsl = slice(half * H, (half + 1) * H)
                            for k in range(8):
                                nc.tensor.matmul(
                                    acc[:, hsl], xhs[k][:, bsl], w_sb[k][:, gsl],
                                    start=(k == 0), stop=(k == 7),
                                )
                        nc.vector.tensor_add(ga[:, csl], acc[:], bb[:, csl])
                    csl = slice(6 * H, 7 * H)
                    acc = pps.tile([P, H], F32, tag="accs")
                    for k in range(8):
                        nc.tensor.matmul(
                            acc[:], xhs[k][:, bsl], w_sb[k][:, csl],
                            start=(k == 0), stop=(k == 7),
                        )
                    nc.vector.tensor_add(ga[:, csl], acc[:], bb[:, csl])

                    # permuted gate slices of ga
                    Z = ga[:, 0 * H:1 * H]
                    D = ga[:, 1 * H:2 * H]
                    Ii = ga[:, 2 * H:3 * H]
                    F = ga[:, 3 * H:4 * H]
                    O = ga[:, 4 * H:5 * H]
                    IB = ga[:, 5 * H:6 * H]
                    FB = ga[:, 6 * H:7 * H]

                    nc.scalar.activation(Z, Z, AF.Tanh)
                    nc.scalar.activation(stash[:, t, :], D, AF.Sigmoid,
                                         scale=-1.0)
                    nc.scalar.activation(ga[:, 2 * H:], ga[:, 2 * H:], AF.Sigmoid)

                    nc.sync.dma_start(o_d[rows, :], O)

                    nc.vector.tensor_mul(F, F, ctj[:])    # f*ct
                    nc.vector.tensor_mul(Ii, Ii, Z)       # i*z
                    nc.vector.tensor_add(F, F, Ii)        # c
                    nc.sync.dma_start(c_d[rows, :], F)
                    nc.vector.tensor_mul(IB, IB, Z)       # ib*z
                    last_sn = nc.scalar.activation(Z, F, AF.Tanh)  # tanh(c)
                    nc.vector.tensor_mul(FB, FB, ctj[:])  # fb*ct
                    nc.vector.tensor_add(FB, FB, IB)      # cbar
                    nc.sync.dma_start(cb_d[rows, :], FB)
                    nc.vector.tensor_mul(Z, O, Z)         # h = o*tanh(c)
                    nc.sync.dma_start(h_d[rows, :], Z)

            # phase 2: decay_rate = softplus(wd) = -ln(sigmoid(-wd))
            dr_r = dr_d.rearrange("(n t p) c -> n p t c", t=SUP, p=P)
            for chn in range(NSUP):
                chsl = slice(chn * SUP, (chn + 1) * SUP)
                ln = nc.scalar.activation(stash[:, chsl, :], stash[:, chsl, :],
                                          AF.Ln)
                # keep Ln after every main-pass ACT: one table switch total
                add_dep_helper(ln.ins, last_sn.ins, reason="phase2 after phase1")
                nc.vector.tensor_scalar_mul(stash[:, chsl, :], stash[:, chsl, :],
                                            -1.0)
                nc.sync.dma_start(dr_r[chn], stash[:, chsl, :])

    nc.compile()
    return nc




def kernel(x, ht, ct, Wx, bx, Wh, bh):
    global _nc_cache, LAST_RESULTS
    if _nc_cache is None:
        _nc_cache = _build()
    nc = _nc_cache

    x = np.ascontiguousarray(x, dtype=np.float32)
    ht = np.ascontiguousarray(ht, dtype=np.float32)
    ct = np.ascontiguousarray(ct, dtype=np.float32)

    # host staging: transpose/concat/cast + gate permutation + bias broadcast
    xh_full = np.empty((K2, B), dtype=NPBF16)
    xh_full[:I, :] = x.T.astype(NPBF16)
    xh_full[I:, :] = ht.T.astype(NPBF16)

    WxT = np.asarray(Wx, dtype=np.float32).T   # [512, 3584]
    WhT = np.asarray(Wh, dtype=np.float32).T
    bsum = np.asarray(bx, dtype=np.float32) + np.asarray(bh, dtype=np.float32)
    w2 = np.empty((K2, G), dtype=NPBF16)
    bbp = np.empty(G, dtype=np.float32)
    for n, old in enumerate(PERM):
        dsl = slice(n * H, (n + 1) * H)
        ssl = slice(old * H, (old + 1) * H)
        w2[:I, dsl] = WxT[:, ssl].astype(NPBF16)
        w2[I:, dsl] = WhT[:, ssl].astype(NPBF16)
        bbp[dsl] = bsum[ssl]
    bb = np.ascontiguousarray(np.broadcast_to(bbp[None, :], (P, G)))

    in_maps = []
    for cidx in range(NCORES):
        sl = slice(cidx * BS, (cidx + 1) * BS)
        in_maps.append({
            "xh": np.ascontiguousarray(xh_full[:, sl]),
            "w2": w2,
            "ct": ct[sl],
            "bb": bb,
        })

    res = run_bass_kernel_spmd(nc, in_maps, core_ids=list(range(NCORES)),
                               trace=TRACE)
    LAST_RESULTS = res

    outs = {}
    for name in ("h", "c", "cb", "o", "dr"):
        outs[name] = np.concatenate(
            [res.results[cidx][name] for cidx in range(NCORES)], axis=0
        )
    return outs["h"], outs["c"], outs["cb"], outs["o"], outs["dr"]



# revision 5
# speedup vs baseline: 1.2058x; 1.2058x over previous
"""CTLSTM cell fused kernel for 8 Trainium2 NeuronCores.

Strategy (data-parallel over batch, TRANSPOSED compute):
  - B=16384 rows sharded 2048/core; weights replicated.
  - Compute g.T: gates on SBUF partitions, batch on the free dim.
    Stationary operand = weight tile [K=128, 128 gates]; moving operand =
    xh [K=128, 1024 batch] bf16 (max bf16 moving free dim). Each PSUM tile
    is one gate-tile x batch-half: [128, 1024] fp32 (2 banks).
  - With gates on partitions the bias is per-partition: the ACT engine
    drains PSUM directly with out = act(psum*scale + bias[p]) in ONE op -
    no DVE bias-add drain at all. DVE only runs the elementwise chain.
  - Gate-group exec order [d, z, i, f, ib, fb, o]:
      * d first: softplus(wd) = -ln(sigmoid(-wd)). The Ln burst (one act
        table switch to natural_log and back) slots in right after the
        4 d-tiles of each half, mid-stream where ACT has slack.
      * o last: the tail after the final matmul is just sigmoid(o) +
        h = o*tanh(c) + store.
  - DMA priority: xh half-0 chunks + first weight tiles first (weights
    staged host-side as contiguous 256 KiB tiles in exec order) so the
    PE starts ~14 us in; weight arrival then stays ahead of the PE.
  - Outputs h/o/dr stored bf16 (halved write traffic), c/cb fp32.
    Host transposes back and upcasts.
"""

import numpy as np
import ml_dtypes

import concourse.bacc as bacc
import concourse.bass as bass
import concourse.mybir as mybir
import concourse.tile as tile
from concourse.bass_utils import run_bass_kernel_spmd

NCORES = 8
B = 16384
I = 512
H = 512
NG = 7
G = NG * H          # 3584
K2 = I + H          # 1024
P = 128
BS = B // NCORES    # 2048 batch cols per core
NH = 2              # batch halves of 1024
BN = BS // NH       # 1024
NQ = H // P         # 4 hidden quadrants (128 gate rows each)
NK = K2 // P        # 8 contraction chunks
NGT = G // P        # 28 gate tiles

BF16 = mybir.dt.bfloat16
F32 = mybir.dt.float32
AF = mybir.ActivationFunctionType
NPBF16 = ml_dtypes.bfloat16

# gate-group exec order: d, z, i, f, ib, fb, o
# reference row order is   i, f, z, o, d, ib, fb
SRC = [4, 2, 0, 1, 5, 6, 3]
GD, GZ, GI, GF, GIB, GFB, GO = range(7)

TRACE = False
LAST_RESULTS = None

_nc_cache = None


def _build():
    nc = bacc.Bacc("TRN2", target_bir_lowering=False, debug=False)

    xh_d = nc.dram_tensor("xh", [NH, NK, P, BN], BF16, kind="ExternalInput")
    w_d = nc.dram_tensor("w", [NGT, P, NK * P], BF16, kind="ExternalInput")
    ct_d = nc.dram_tensor("ct", [H, BS], BF16, kind="ExternalInput")
    bb_d = nc.dram_tensor("bb", [P, NGT], F32, kind="ExternalInput")

    h_d = nc.dram_tensor("h", [H, BS], BF16, kind="ExternalOutput")
    c_d = nc.dram_tensor("c", [H, BS], F32, kind="ExternalOutput")
    cb_d = nc.dram_tensor("cb", [H, BS], F32, kind="ExternalOutput")
    o_d = nc.dram_tensor("o", [H, BS], BF16, kind="ExternalOutput")
    dr_d = nc.dram_tensor("dr", [H, BS], BF16, kind="ExternalOutput")

    with tile.TileContext(nc) as tc:
        with (
            tc.tile_pool(name="wp", bufs=1) as wp,
            tc.tile_pool(name="xp", bufs=1) as xp,
            tc.tile_pool(name="cp", bufs=1) as cp,
            tc.tile_pool(name="gp", bufs=1) as gp,
            tc.tile_pool(name="dp", bufs=1) as dp,
            tc.tile_pool(name="op", bufs=2) as op_,
            tc.tile_pool(name="pp", bufs=4, space=bass.MemorySpace.PSUM) as pp,
        ):
            # --- input DMA issue order = arrival priority ---
            bb = cp.tile([P, NGT], F32, tag="bb")
            nc.sync.dma_start(bb[:], bb_d[:])

            xh_sb = {}
            w_sb = [None] * NGT

            def load_xh(h, k):
                t = xp.tile([P, BN], BF16, tag=f"xh{h}_{k}")
                nc.sync.dma_start(t[:], xh_d[h, k])
                xh_sb[(h, k)] = t

            def load_w(gt):
                t = wp.tile([P, NK * P], BF16, tag=f"w{gt}")
                nc.sync.dma_start(t[:], w_d[gt])
                w_sb[gt] = t

            load_xh(0, 0)
            load_w(0)
            for k in range(1, NK):
                load_xh(0, k)
            for j in range(1, NK):
                load_w(j)
                load_xh(1, j - 1)
            load_xh(1, NK - 1)
            for j in range(NK, 20):
                load_w(j)
            ct_sb = []
            for q in range(NQ):
                t = cp.tile([P, BS], BF16, tag=f"ct{q}")
                nc.sync.dma_start(t[:], ct_d[q * P:(q + 1) * P, :])
                ct_sb.append(t)
            for j in range(20, NGT):
                load_w(j)

            # --- main loop: halves x gate-groups x quadrants ---
            for h in range(NH):
                col = slice(h * BN, (h + 1) * BN)
                sdt = [None] * NQ     # sigmoid(-wd) tiles
                gz = [None] * NQ
                gi = [None] * NQ
                gf = [None] * NQ
                gib = [None] * NQ
                gfb = [None] * NQ
                th = [None] * NQ

                def mm(gt):
                    # moving free dim caps at 512: two accumulation groups
                    # into the two banks of one [128, 1024] PSUM tile
                    acc = pp.tile([P, BN], F32, tag="acc")
                    for bh in range(2):
                        bsl = slice(bh * 512, (bh + 1) * 512)
                        for k in range(NK):
                            nc.tensor.matmul(
                                acc[:, bsl], w_sb[gt][:, k * P:(k + 1) * P],
                                xh_sb[(h, k)][:, bsl],
                                start=(k == 0), stop=(k == NK - 1),
                            )
                    return acc

                for grp in range(7):
                    for q in range(NQ):
                        gt = grp * NQ + q
                        rows = slice(q * P, (q + 1) * P)
                        acc = mm(gt)
                        bias = bb[:, gt:gt + 1]
                        if grp == GD:
                            # sigmoid(-(psum+b)) ; d-bias staged negated
                            sd = dp.tile([P, BN], BF16, tag=f"sd{q}")
                            nc.scalar.activation(sd[:], acc[:], AF.Sigmoid,
                                                 bias=bias, scale=-1.0)
                            sdt[q] = sd
                        elif grp == GZ:
                            g = gp.tile([P, BN], BF16, tag=f"z{q}")
                            nc.scalar.activation(g[:], acc[:], AF.Tanh,
                                                 bias=bias)
                            gz[q] = g
                        else:
                            tagn = ("", "", "i", "f", "ib", "fb", "o")[grp]
                            g = gp.tile([P, BN], BF16, tag=f"{tagn}{q}")
                            nc.scalar.activation(g[:], acc[:], AF.Sigmoid,
                                                 bias=bias)
                            if grp == GI:
                                gi[q] = g
                            elif grp == GF:
                                gf[q] = g
                            elif grp == GIB:
                                gib[q] = g
                            elif grp == GFB:
                                gfb[q] = g
                                if q == NQ - 1:
                                    # chain part A for all quadrants
                                    for qq in range(NQ):
                                        rr = slice(qq * P, (qq + 1) * P)
                                        ctq = ct_sb[qq][:, col]
                                        c = op_.tile([P, BN], F32, tag="c")
                                        tmp = op_.tile([P, BN], F32, tag="tmp")
                                        cb = op_.tile([P, BN], F32, tag="cb")
                                        # all 4 live until chain B: bufs=4
                                        t_ = op_.tile([P, BN], BF16, tag="th",
                                                      bufs=4)
                                        nc.vector.tensor_mul(c[:], gf[qq][:], ctq)
                                        nc.vector.tensor_mul(tmp[:], gi[qq][:], gz[qq][:])
                                        nc.vector.tensor_add(c[:], c[:], tmp[:])
                                        nc.sync.dma_start(c_d[rr, col], c[:])
                                        nc.scalar.activation(t_[:], c[:], AF.Tanh)
                                        th[qq] = t_
                                        nc.vector.tensor_mul(cb[:], gfb[qq][:], ctq)
                                        nc.vector.tensor_mul(tmp[:], gib[qq][:], gz[qq][:])
                                        nc.vector.tensor_add(cb[:], cb[:], tmp[:])
                                        nc.sync.dma_start(cb_d[rr, col], cb[:])
                            else:  # GO: chain part B
                                nc.sync.dma_start(o_d[rows, col], g[:])
                                hh = op_.tile([P, BN], BF16, tag="hh")
                                nc.vector.tensor_mul(hh[:], g[:], th[q][:])
                                nc.sync.dma_start(h_d[rows, col], hh[:])
                    if grp == GD:
                        # softplus tail for this half: -ln(sigmoid(-wd))
                        for q in range(NQ):
                            rows = slice(q * P, (q + 1) * P)
                            nc.scalar.activation(sdt[q][:], sdt[q][:], AF.Ln)
                            nc.vector.tensor_scalar_mul(sdt[q][:], sdt[q][:],
                                                        -1.0)
                            nc.sync.dma_start(dr_d[rows, col], sdt[q][:])

    nc.compile()
    return nc


def kernel(x, ht, ct, Wx, bx, Wh, bh):
    global _nc_cache, LAST_RESULTS
    if _nc_cache is None:
        _nc_cache = _build()
    nc = _nc_cache

    x = np.ascontiguousarray(x, dtype=np.float32)
    ht = np.ascontiguousarray(ht, dtype=np.float32)
    ct = np.ascontiguousarray(ct, dtype=np.float32)

    # weights: [K2, G] in exec gate order, tiled [28][128][8*128]
    WxT = np.asarray(Wx, dtype=np.float32).T   # [512, 3584]
    WhT = np.asarray(Wh, dtype=np.float32).T
    bsum = np.asarray(bx, dtype=np.float32) + np.asarray(bh, dtype=np.float32)
    w2 = np.empty((K2, G), dtype=NPBF16)
    bbp = np.empty(G, dtype=np.float32)
    for n, old in enumerate(SRC):
        dsl = slice(n * H, (n + 1) * H)
        ssl = slice(old * H, (old + 1) * H)
        w2[:I, dsl] = WxT[:, ssl].astype(NPBF16)
        w2[I:, dsl] = WhT[:, ssl].astype(NPBF16)
        bbp[dsl] = bsum[ssl]
    bbp[0:H] = -bbp[0:H]           # d-gate bias negated (scale=-1 trick)
    # w_stage[gt, p, k*128+g] = w2[k*128+p, gt*128+g]
    w_stage = np.ascontiguousarray(
        w2.reshape(NK, P, NGT, P).transpose(2, 1, 0, 3).reshape(NGT, P, NK * P)
    )
    bbT = np.ascontiguousarray(bbp.reshape(NGT, P).T)   # [128, 28]

    in_maps = []
    for cidx in range(NCORES):
        sl = slice(cidx * BS, (cidx + 1) * BS)
        xh_full = np.empty((K2, BS), dtype=NPBF16)
        xh_full[:I, :] = x[sl].T.astype(NPBF16)
        xh_full[I:, :] = ht[sl].T.astype(NPBF16)
        # [2, 8, 128, 1024] halves-major
        xh_stage = np.ascontiguousarray(
            xh_full.reshape(NK, P, NH, BN).transpose(2, 0, 1, 3)
        )
        ctT = np.ascontiguousarray(ct[sl].T.astype(NPBF16))
        in_maps.append({
            "xh": xh_stage,
            "w": w_stage,
            "ct": ctT,
            "bb": bbT,
        })

    res = run_bass_kernel_spmd(nc, in_maps, core_ids=list(range(NCORES)),
                               trace=TRACE)
    LAST_RESULTS = res

    outs = {}
    for name in ("h", "c", "cb", "o", "dr"):
        full = np.concatenate(
            [res.results[cidx][name] for cidx in range(NCORES)], axis=1
        )
        outs[name] = np.ascontiguousarray(full.T.astype(np.float32))
    return outs["h"], outs["c"], outs["cb"], outs["o"], outs["dr"]


# revision 13
# speedup vs baseline: 1.2227x; 1.0140x over previous
"""CTLSTM cell fused kernel for 8 Trainium2 NeuronCores.

Strategy (data-parallel over batch, TRANSPOSED compute):
  - B=16384 rows sharded 2048/core; weights replicated.
  - Compute g.T: gates on SBUF partitions, batch on the free dim.
    Stationary operand = weight tile [K=128, 128 gates]; moving operand =
    xh [K=128, 1024 batch] bf16 (max bf16 moving free dim). Each PSUM tile
    is one gate-tile x batch-half: [128, 1024] fp32 (2 banks).
  - With gates on partitions the bias is per-partition: the ACT engine
    drains PSUM directly with out = act(psum*scale + bias[p]) in ONE op -
    no DVE bias-add drain at all. DVE only runs the elementwise chain.
  - Gate-group exec order [d, z, i, f, ib, fb, o]:
      * d first: softplus(wd) = -ln(sigmoid(-wd)). The Ln burst (one act
        table switch to natural_log and back) slots in right after the
        4 d-tiles of each half, mid-stream where ACT has slack.
      * o last: the tail after the final matmul is just sigmoid(o) +
        h = o*tanh(c) + store.
  - DMA priority: xh half-0 chunks + first weight tiles first (weights
    staged host-side as contiguous 256 KiB tiles in exec order) so the
    PE starts ~14 us in; weight arrival then stays ahead of the PE.
  - Outputs h/o/dr stored bf16 (halved write traffic), c/cb fp32.
    Host transposes back and upcasts.
"""

import numpy as np
import ml_dtypes

import concourse.bacc as bacc
import concourse.bass as bass
import concourse.mybir as mybir
import concourse.tile as tile
from concourse.bass_utils import run_bass_kernel_spmd

NCORES = 8
B = 16384
I = 512
H = 512
NG = 7
G = NG * H          # 3584
K2 = I + H          # 1024
P = 128
BS = B // NCORES    # 2048 batch cols per core
NH = 2              # batch halves of 1024
BN = BS // NH       # 1024
NQ = H // P         # 4 hidden quadrants (128 gate rows each)
NK = K2 // P        # 8 contraction chunks
NGT = G // P        # 28 gate tiles

BF16 = mybir.dt.bfloat16
F32 = mybir.dt.float32
AF = mybir.ActivationFunctionType
NPBF16 = ml_dtypes.bfloat16

# gate-group exec order: d, z, i, f, ib, fb, o
# reference row order is   i, f, z, o, d, ib, fb
SRC = [4, 2, 0, 1, 5, 6, 3]
GD, GZ, GI, GF, GIB, GFB, GO = range(7)

TRACE = False
LAST_RESULTS = None

_nc_cache = None


def _build():
    nc = bacc.Bacc("TRN2", target_bir_lowering=False, debug=False)

    xh_d = nc.dram_tensor("xh", [NH, NK, P, BN], BF16, kind="ExternalInput")
    w_d = nc.dram_tensor("w", [NGT, P, NK * P], BF16, kind="ExternalInput")
    ct_d = nc.dram_tensor("ct", [H, BS], BF16, kind="ExternalInput")
    bb_d = nc.dram_tensor("bb", [P, NGT], F32, kind="ExternalInput")

    h_d = nc.dram_tensor("h", [H, BS], BF16, kind="ExternalOutput")
    c_d = nc.dram_tensor("c", [H, BS], F32, kind="ExternalOutput")
    cb_d = nc.dram_tensor("cb", [H, BS], F32, kind="ExternalOutput")
    o_d = nc.dram_tensor("o", [H, BS], BF16, kind="ExternalOutput")
    dr_d = nc.dram_tensor("dr", [H, BS], BF16, kind="ExternalOutput")

    with tile.TileContext(nc) as tc:
        with (
            tc.tile_pool(name="wp", bufs=1) as wp,
            tc.tile_pool(name="xp", bufs=1) as xp,
            tc.tile_pool(name="cp", bufs=1) as cp,
            tc.tile_pool(name="gp", bufs=1) as gp,
            tc.tile_pool(name="dp", bufs=1) as dp,
            tc.tile_pool(name="op", bufs=2) as op_,
            tc.tile_pool(name="pp", bufs=4, space=bass.MemorySpace.PSUM) as pp,
        ):
            # --- input DMA issue order = arrival priority ---
            bb = cp.tile([P, NGT], F32, tag="bb")
            nc.sync.dma_start(bb[:], bb_d[:])

            xh_sb = {}
            w_sb = [None] * NGT

            def load_xh(h, k):
                t = xp.tile([P, BN], BF16, tag=f"xh{h}_{k}")
                nc.sync.dma_start(t[:], xh_d[h, k])
                xh_sb[(h, k)] = t

            def load_w(gt):
                t = wp.tile([P, NK * P], BF16, tag=f"w{gt}")
                nc.sync.dma_start(t[:], w_d[gt])
                w_sb[gt] = t

            load_xh(0, 0)
            load_w(0)
            for k in range(1, NK):
                load_xh(0, k)
            for j in range(1, NK):
                load_w(j)
            for j in range(NK, 2 * NK):
                load_w(j)
                load_xh(1, j - NK)
            for j in range(2 * NK, 20):
                load_w(j)
            ct_sb = []
            for q in range(NQ):
                t = cp.tile([P, BS], BF16, tag=f"ct{q}")
                nc.sync.dma_start(t[:], ct_d[q * P:(q + 1) * P, :])
                ct_sb.append(t)
            for j in range(20, NGT):
                load_w(j)

            # --- main loop: halves x gate-groups x quadrants ---
            for h in range(NH):
                col = slice(h * BN, (h + 1) * BN)
                # one [128, 4096] sigmoid(-wd) supertile per half: the Ln
                # is then a single ACT op, so the scheduler cannot
                # interleave it with sigmoid drains (one table switch
                # each way per half instead of per-quadrant)
                sdt = dp.tile([P, NQ * BN], BF16, tag="sd")
                gz = [None] * NQ
                gi = [None] * NQ
                gf = [None] * NQ
                gib = [None] * NQ
                gfb = [None] * NQ
                th = [None] * NQ

                def mm(gt, bank_outer=False):
                    # moving free dim caps at 512: two accumulation groups
                    # into the two banks of one [128, 1024] PSUM tile.
                    # k-outer/bank-inner loads each stationary tile once;
                    # bank-outer (last tile) lets bank 0 drain while bank 1
                    # is still accumulating.
                    acc = pp.tile([P, BN], F32, tag="acc")
                    loops = ([(bh, k) for bh in range(2) for k in range(NK)]
                             if bank_outer else
                             [(bh, k) for k in range(NK) for bh in range(2)])
                    for bh, k in loops:
                        bsl = slice(bh * 512, (bh + 1) * 512)
                        nc.tensor.matmul(
                            acc[:, bsl], w_sb[gt][:, k * P:(k + 1) * P],
                            xh_sb[(h, k)][:, bsl],
                            start=(k == 0), stop=(k == NK - 1),
                        )
                    return acc

                for grp in range(7):
                    for q in range(NQ):
                        gt = grp * NQ + q
                        rows = slice(q * P, (q + 1) * P)
                        last = (grp == GO and h == NH - 1 and q == NQ - 1)
                        acc = mm(gt, bank_outer=last)
                        bias = bb[:, gt:gt + 1]
                        if grp == GD:
                            # sigmoid(-(psum+b)) ; d-bias staged negated
                            nc.scalar.activation(
                                sdt[:, q * BN:(q + 1) * BN], acc[:],
                                AF.Sigmoid, bias=bias, scale=-1.0)
                        elif grp == GZ:
                            g = gp.tile([P, BN], BF16, tag=f"z{q}")
                            nc.scalar.activation(g[:], acc[:], AF.Tanh,
                                                 bias=bias)
                            gz[q] = g
                        elif last:
                            # final tile: drain per bank so the tail after
                            # the very last matmul is one 512-col chain
                            g = gp.tile([P, BN], BF16, tag=f"o{q}")
                            hh = op_.tile([P, BN], BF16, tag="hh")
                            for b2 in range(2):
                                ssl = slice(b2 * 512, (b2 + 1) * 512)
                                csl = slice(h * BN + b2 * 512,
                                            h * BN + (b2 + 1) * 512)
                                nc.scalar.activation(g[:, ssl], acc[:, ssl],
                                                     AF.Sigmoid, bias=bias)
                                nc.sync.dma_start(o_d[rows, csl], g[:, ssl])
                                nc.vector.tensor_mul(hh[:, ssl], g[:, ssl],
                                                     th[q][:, ssl])
                                nc.sync.dma_start(h_d[rows, csl], hh[:, ssl])
                        else:
                            tagn = ("", "", "i", "f", "ib", "fb", "o")[grp]
                            g = gp.tile([P, BN], BF16, tag=f"{tagn}{q}")
                            nc.scalar.activation(g[:], acc[:], AF.Sigmoid,
                                                 bias=bias)
                            if grp == GI:
                                gi[q] = g
                            elif grp == GF:
                                gf[q] = g
                            elif grp == GIB:
                                gib[q] = g
                            elif grp == GFB:
                                gfb[q] = g
                                if q == NQ - 1:
                                    # chain part A for all quadrants
                                    for qq in range(NQ):
                                        rr = slice(qq * P, (qq + 1) * P)
                                        ctq = ct_sb[qq][:, col]
                                        c = op_.tile([P, BN], F32, tag="c")
                                        tmp = op_.tile([P, BN], F32, tag="tmp")
                                        cb = op_.tile([P, BN], F32, tag="cb")
                                        # all 4 live until chain B: bufs=4
                                        t_ = op_.tile([P, BN], BF16, tag="th",
                                                      bufs=4)
                                        nc.vector.tensor_mul(c[:], gf[qq][:], ctq)
                                        nc.vector.tensor_mul(tmp[:], gi[qq][:], gz[qq][:])
                                        nc.vector.tensor_add(c[:], c[:], tmp[:])
                                        nc.sync.dma_start(c_d[rr, col], c[:])
                                        nc.scalar.activation(t_[:], c[:], AF.Tanh)
                                        th[qq] = t_
                                        nc.vector.tensor_mul(cb[:], gfb[qq][:], ctq)
                                        nc.vector.tensor_mul(tmp[:], gib[qq][:], gz[qq][:])
                                        nc.vector.tensor_add(cb[:], cb[:], tmp[:])
                                        nc.sync.dma_start(cb_d[rr, col], cb[:])
                            else:  # GO: chain part B
                                nc.sync.dma_start(o_d[rows, col], g[:])
                                hh = op_.tile([P, BN], BF16, tag="hh")
                                nc.vector.tensor_mul(hh[:], g[:], th[q][:])
                                nc.sync.dma_start(h_d[rows, col], hh[:])
                    if grp == GD:
                        # softplus for this half: -ln(sigmoid(-wd)),
                        # single Ln + negate over the supertile
                        nc.scalar.activation(sdt[:], sdt[:], AF.Ln)
                        nc.vector.tensor_scalar_mul(sdt[:], sdt[:], -1.0)
                        for q in range(NQ):
                            rows = slice(q * P, (q + 1) * P)
                            nc.sync.dma_start(dr_d[rows, col],
                                              sdt[:, q * BN:(q + 1) * BN])

    nc.compile()
    return nc


def kernel(x, ht, ct, Wx, bx, Wh, bh):
    global _nc_cache, LAST_RESULTS
    if _nc_cache is None:
        _nc_cache = _build()
    nc = _nc_cache

    x = np.ascontiguousarray(x, dtype=np.float32)
    ht = np.ascontiguousarray(ht, dtype=np.float32)
    ct = np.ascontiguousarray(ct, dtype=np.float32)

    # weights: [K2, G] in exec gate order, tiled [28][128][8*128]
    WxT = np.asarray(Wx, dtype=np.float32).T   # [512, 3584]
    WhT = np.asarray(Wh, dtype=np.float32).T
    bsum = np.asarray(bx, dtype=np.float32) + np.asarray(bh, dtype=np.float32)
    w2 = np.empty((K2, G), dtype=NPBF16)
    bbp = np.empty(G, dtype=np.float32)
    for n, old in enumerate(SRC):
        dsl = slice(n * H, (n + 1) * H)
        ssl = slice(old * H, (old + 1) * H)
        w2[:I, dsl] = WxT[:, ssl].astype(NPBF16)
        w2[I:, dsl] = WhT[:, ssl].astype(NPBF16)
        bbp[dsl] = bsum[ssl]
    bbp[0:H] = -bbp[0:H]           # d-gate bias negated (scale=-1 trick)
    # w_stage[gt, p, k*128+g] = w2[k*128+p, gt*128+g]
    w_stage = np.ascontiguousarray(
        w2.reshape(NK, P, NGT, P).transpose(2, 1, 0, 3).reshape(NGT, P, NK * P)
    )
    bbT = np.ascontiguousarray(bbp.reshape(NGT, P).T)   # [128, 28]

    in_maps = []
    for cidx in range(NCORES):
        sl = slice(cidx * BS, (cidx + 1) * BS)
        xh_full = np.empty((K2, BS), dtype=NPBF16)
        xh_full[:I, :] = x[sl].T.astype(NPBF16)
        xh_full[I:, :] = ht[sl].T.astype(NPBF16)
        # [2, 8, 128, 1024] halves-major
        xh_stage = np.ascontiguousarray(
            xh_full.reshape(NK, P, NH, BN).transpose(2, 0, 1, 3)
        )
        ctT = np.ascontiguousarray(ct[sl].T.astype(NPBF16))
        in_maps.append({
            "xh": xh_stage,
            "w": w_stage,
            "ct": ctT,
            "bb": bbT,
        })

    res = run_bass_kernel_spmd(nc, in_maps, core_ids=list(range(NCORES)),
                               trace=TRACE)
    LAST_RESULTS = res

    outs = {}
    for name in ("h", "c", "cb", "o", "dr"):
        full = np.concatenate(
            [res.results[cidx][name] for cidx in range(NCORES)], axis=1
        )
        outs[name] = np.ascontiguousarray(full.T.astype(np.float32))
    return outs["h"], outs["c"], outs["cb"], outs["o"], outs["dr"]
